# revision 1
# baseline (speedup 1.0000x reference)
"""Fused LayerNorm + multi-head self-attention (diagonal-masked) + out-projection
for Trainium2, SPMD across 8 NeuronCores.

Sharding: batch (2) x head-groups (4 groups of 4 heads) = 8 shards.
Each core: LN(x_b) -> QKV for its 4 heads -> full [n,n] attention -> partial
out-projection with its w_out row-slice. Host sums the 4 partials per batch.

Hardcoded problem shape: x [2, 2048, 1024], 16 heads, dim_head 64.
"""
import sys

sys.path.insert(0, "/opt/trn_rl_repo")

import numpy as np

import concourse.bass as bass
import concourse.bacc as bacc
import concourse.tile as tile
from concourse import mybir

B = 2
N = 2048
D = 1024
HEADS = 16
DH = 64
H_LOC = 4            # heads per core
M_LOC = H_LOC * DH   # 256: local inner dim
SCALE = DH ** -0.5
LN_EPS = 1e-5
MASK_VALUE = float(-np.finfo(np.float32).max)
NT = N // 128        # 16 row tiles
DC = D // 128        # 8 d-model chunks
F32 = mybir.dt.float32


def build_program_v2(apply_gamma_beta: bool, reps: int = 1):
    """Minimal-instruction variant: x arrives pre-transposed [D, N], LN stats
    via gpsimd partition_all_reduce, batched evictions, pair-combined score
    tiles, denominator folded into the PV matmul as a 65th ones-column."""
    from contextlib import ExitStack
    nc = bacc.Bacc("TRN2", target_bir_lowering=False, debug=False)

    x_t = nc.dram_tensor("xt_s", [D, N], F32, kind="ExternalInput")
    wqk_t = nc.dram_tensor("wqk_s", [D, 2 * M_LOC], F32, kind="ExternalInput")
    wv_t = nc.dram_tensor("wv_s", [D, M_LOC], F32, kind="ExternalInput")
    wo_t = nc.dram_tensor("wo_s", [M_LOC, D], F32, kind="ExternalInput")
    g_t = nc.dram_tensor("gamma_s", [D], F32, kind="ExternalInput")
    b_t = nc.dram_tensor("beta_s", [D], F32, kind="ExternalInput")
    out_t = nc.dram_tensor("out_s", [N, D], F32, kind="ExternalOutput")

    with tile.TileContext(nc) as tc:
        with ExitStack() as ctx:
            const = ctx.enter_context(tc.tile_pool(name="const", bufs=1))
            maskneg = const.tile([128, 128], F32)
            nc.gpsimd.memset(maskneg, 0.0)
            nc.gpsimd.affine_select(
                out=maskneg, in_=maskneg,
                compare_op=mybir.AluOpType.not_equal,
                fill=MASK_VALUE, base=0, pattern=[[-1, 128]],
                channel_multiplier=1)
            eps_col = const.tile([128, 1], F32)
            nc.vector.memset(eps_col, LN_EPS)
            identity = const.tile([128, 128], F32)
            from concourse.masks import make_identity
            make_identity(nc, identity)
            gammab = betab = None
            if apply_gamma_beta:
                gammab = const.tile([128, DC], F32)
                betab = const.tile([128, DC], F32)
                # gamma[d] -> [p, c] with d = c*128 + p
                nc.sync.dma_start(
                    gammab, bass.AP(tensor=g_t, offset=0,
                                    ap=[[1, 128], [128, DC]]))
                nc.sync.dma_start(
                    betab, bass.AP(tensor=b_t, offset=0,
                                   ap=[[1, 128], [128, DC]]))

            wpool = ctx.enter_context(tc.tile_pool(name="w", bufs=1))
            wqk = wpool.tile([128, DC, 2 * M_LOC], F32)
            nc.sync.dma_start(
                wqk, bass.AP(tensor=wqk_t, offset=0,
                             ap=[[2 * M_LOC, 128], [128 * 2 * M_LOC, DC],
                                 [1, 2 * M_LOC]]))
            wv = wpool.tile([128, DC, M_LOC], F32)
            nc.sync.dma_start(
                wv, bass.AP(tensor=wv_t, offset=0,
                            ap=[[M_LOC, 128], [128 * M_LOC, DC], [1, M_LOC]]))
            wo = wpool.tile([128, 2, D], F32)
            nc.sync.dma_start(
                wo, bass.AP(tensor=wo_t, offset=0,
                            ap=[[D, 128], [128 * D, 2], [1, D]]))

            for rep in range(reps):
                body_v2(nc, tc, rep, x_t, out_t, maskneg, eps_col, identity,
                        wqk, wv, wo, gammab, betab)

    nc.compile()
    return nc


def body_v2(nc, tc, rep, x_t, out_t, maskneg, eps_col, identity, wqk, wv, wo,
            gammab, betab):
    from contextlib import ExitStack
    r = rep

    with ExitStack() as ctx:
        qkT_pool = ctx.enter_context(tc.tile_pool(name=f"qkT{r}", bufs=4))
        vpool = ctx.enter_context(tc.tile_pool(name=f"v{r}", bufs=1))
        qkT = [qkT_pool.tile([128, N], F32, tag="qkT", name=f"qkT{i}")
               for i in range(4)]
        v_big = vpool.tile([128, NT, H_LOC * 65], F32)

        # ---- load x transposed + LayerNorm (stats via partition reduce) ----
        with ExitStack() as ctxA:
            xpool = ctxA.enter_context(tc.tile_pool(name=f"xT{r}", bufs=1))
            xT = xpool.tile([128, DC, N], F32)
            nc.sync.dma_start(
                xT, bass.AP(tensor=x_t, offset=0,
                            ap=[[N, 128], [128 * N, DC], [1, N]]))
            ctxR = ExitStack()
            rpool = ctxR.enter_context(tc.tile_pool(name=f"rows{r}", bufs=1))
            P = rpool.tile([128, 2, N], F32)
            T4 = rpool.tile([128, 2, N], F32)
            mub = rpool.tile([128, N], F32)
            rstdb = rpool.tile([128, N], F32)
            # chunk-pair accumulation: contiguous FD=4096 ops (probe-verified
            # near-flat cost); P is reused for the squares pass after s1
            import concourse.bass_isa as bass_isa
            nc.vector.tensor_copy(P, xT[:, 0:2, :])
            for g in range(1, DC // 2):
                nc.vector.tensor_add(P, P, xT[:, 2 * g:2 * g + 2, :])
            nc.vector.tensor_add(P[:, 0, :], P[:, 0, :], P[:, 1, :])
            nc.gpsimd.partition_all_reduce(
                mub, P[:, 0, :], channels=128, reduce_op=bass_isa.ReduceOp.add)
            nc.vector.tensor_mul(P, xT[:, 0:2, :], xT[:, 0:2, :])
            for g in range(1, DC // 2):
                nc.vector.tensor_mul(T4, xT[:, 2 * g:2 * g + 2, :],
                                     xT[:, 2 * g:2 * g + 2, :])
                nc.vector.tensor_add(P, P, T4)
            nc.vector.tensor_add(P[:, 0, :], P[:, 0, :], P[:, 1, :])
            nc.gpsimd.partition_all_reduce(
                rstdb, P[:, 0, :], channels=128, reduce_op=bass_isa.ReduceOp.add)
            nc.vector.tensor_scalar_mul(mub, mub, 1.0 / D)
            # T4[:,0] <- -mu^2 ; rstdb <- E[x^2] ; var = rstdb - mu^2
            nc.vector.scalar_tensor_tensor(
                out=T4[:, 0, :], in0=mub, scalar=-1.0, in1=mub,
                op0=mybir.AluOpType.mult, op1=mybir.AluOpType.mult)
            nc.vector.tensor_scalar_mul(rstdb, rstdb, 1.0 / D)
            nc.vector.tensor_add(rstdb, rstdb, T4[:, 0, :])
            nc.scalar.activation(rstdb, rstdb,
                                 mybir.ActivationFunctionType.Sqrt,
                                 bias=eps_col, scale=1.0)
            nc.vector.reciprocal(rstdb, rstdb)
            # normalize xT in place per chunk
            for c in range(DC):
                xv = xT[:, c, :]
                nc.vector.tensor_sub(xv, xv, mub)
                nc.vector.tensor_mul(xv, xv, rstdb)
            ctxR.close()
            if gammab is not None:
                for c in range(DC):
                    xv = xT[:, c, :]
                    nc.vector.tensor_scalar(
                        out=xv, in0=xv,
                        scalar1=gammab[:, c:c + 1], scalar2=betab[:, c:c + 1],
                        op0=mybir.AluOpType.mult, op1=mybir.AluOpType.add)

            # ---- QKV ----
            with ExitStack() as ctx2:
                psq = ctx2.enter_context(
                    tc.tile_pool(name=f"psQ{r}", bufs=2, space="PSUM"))
                for mi in range(4):
                    ps = psq.tile([128, N], F32, tag="psQ", name=f"psq{mi}")
                    for nt in range(4):
                        for c in range(DC):
                            nc.tensor.matmul(
                                ps[:, nt * 512:(nt + 1) * 512],
                                wqk[:, c, mi * 128:(mi + 1) * 128],
                                xT[:, c, nt * 512:(nt + 1) * 512],
                                start=(c == 0), stop=(c == DC - 1))
                    nc.vector.tensor_copy(qkT[mi], ps)
            with ExitStack() as ctx2:
                # vT = Wv^T @ xn^T at full matmul efficiency, then PE-transpose
                # [128,128] blocks into the row-layout v_big slots.
                vtp = ctx2.enter_context(tc.tile_pool(name=f"vT{r}", bufs=2))
                vT_sb = [vtp.tile([128, N], F32, tag="vT", name=f"vTs{i}")
                         for i in range(2)]
                with ExitStack() as ctx3:
                    psvt = ctx3.enter_context(
                        tc.tile_pool(name=f"psVT{r}", bufs=2, space="PSUM"))
                    for mi in range(2):
                        ps = psvt.tile([128, N], F32, tag="psVT",
                                       name=f"psvt{mi}")
                        for nt in range(4):
                            for c in range(DC):
                                nc.tensor.matmul(
                                    ps[:, nt * 512:(nt + 1) * 512],
                                    wv[:, c, mi * 128:(mi + 1) * 128],
                                    xT[:, c, nt * 512:(nt + 1) * 512],
                                    start=(c == 0), stop=(c == DC - 1))
                        nc.vector.tensor_copy(vT_sb[mi], ps)
                with ExitStack() as ctx3:
                    pst = ctx3.enter_context(
                        tc.tile_pool(name=f"psT{r}", bufs=2, space="PSUM"))
                    for np4 in range(NT // 4):
                        ps = pst.tile([128, 4, M_LOC], F32, tag="psT",
                                      name=f"pst{np4}")
                        for quarter in range(4):
                            nt = np4 * 4 + quarter
                            for mi in range(2):
                                nc.tensor.transpose(
                                    ps[:, quarter,
                                       mi * 128:(mi + 1) * 128],
                                    vT_sb[mi][:, nt * 128:(nt + 1) * 128],
                                    identity)
                        nc.vector.tensor_copy(
                            v_big[:, np4 * 4:np4 * 4 + 4, :].rearrange(
                                "p t (h c) -> p t h c", c=65)[:, :, :, 0:64],
                            ps.rearrange("p t (h c) -> p t h c", c=64))
                ones_cols = v_big.rearrange(
                    "p t (h c) -> p t h c", c=65)[:, :, :, 64:65]
                nc.vector.memset(ones_cols, 1.0)

        # ---- attention ----
        oT_pool = ctx.enter_context(tc.tile_pool(name=f"oT{r}", bufs=2))
        oTp = [oT_pool.tile([128, N], F32, tag="oT", name=f"oTp{p}")
               for p in range(2)]
        with ExitStack() as ctx2:
            psS = ctx2.enter_context(
                tc.tile_pool(name=f"psA{r}", bufs=1, space="PSUM"))
            psO = ctx2.enter_context(
                tc.tile_pool(name=f"psO{r}", bufs=1, space="PSUM"))
            aT_pool = ctx2.enter_context(tc.tile_pool(name=f"aT{r}", bufs=6))
            dpool = ctx2.enter_context(tc.tile_pool(name=f"dn{r}", bufs=2))

            mask3 = bass.AP(tensor=maskneg.tensor, offset=maskneg.offset,
                            ap=[maskneg.ap[0], [0, 2], maskneg.ap[1]])

            for pair in range(2):
                qTt = qkT[pair]
                kTt = qkT[2 + pair]
                for half in range(2):
                    q0 = half * 1024
                    ot_ps2 = psO.tile([65, N], F32, tag="psO", name="otps")
                    for m in range(NT):
                        s_ps = psS.tile([128, N], F32, tag="psS", name="sps")
                        for hh in range(2):
                            pb = hh * 64
                            for nt in range(2):
                                nc.tensor.matmul(
                                    s_ps[:, hh * 1024 + nt * 512:
                                         hh * 1024 + (nt + 1) * 512],
                                    kTt[pb:pb + 64, m * 128:(m + 1) * 128],
                                    qTt[pb:pb + 64,
                                        q0 + nt * 512:q0 + (nt + 1) * 512],
                                    start=True, stop=True)
                        if q0 <= m * 128 < q0 + 1024:
                            off = m * 128 - q0
                            sv = s_ps.rearrange("p (t q) -> p t q",
                                                t=2)[:, :, off:off + 128]
                            nc.vector.tensor_add(sv, sv, mask3)
                        aT = aT_pool.tile([128, N], F32, tag="aT")
                        nc.scalar.activation(
                            aT, s_ps, mybir.ActivationFunctionType.Exp,
                            scale=SCALE)
                        for hh in range(2):
                            h = pair * 2 + hh
                            for nt in range(2):
                                nc.tensor.matmul(
                                    ot_ps2[:, hh * 1024 + nt * 512:
                                           hh * 1024 + (nt + 1) * 512],
                                    v_big[:, m, h * 65:(h + 1) * 65],
                                    aT[:, hh * 1024 + nt * 512:
                                       hh * 1024 + (nt + 1) * 512],
                                    start=(m == 0), stop=(m == NT - 1))
                    recip_row = dpool.tile([1, N], F32, tag="recip")
                    nc.vector.reciprocal(recip_row, ot_ps2[64:65, :])
                    recip_b = dpool.tile([64, N], F32, tag="recipb")
                    nc.gpsimd.partition_broadcast(recip_b, recip_row)
                    nc.vector.tensor_mul(
                        oTp[pair][0:64, q0:q0 + 1024],
                        ot_ps2[0:64, 0:1024], recip_b[:, 0:1024])
                    tmpB = dpool.tile([64, 1024], F32, tag="tmpB")
                    nc.vector.tensor_mul(tmpB, ot_ps2[0:64, 1024:2048],
                                         recip_b[:, 1024:2048])
                    nc.sync.dma_start(
                        oTp[pair][64:128, q0:q0 + 1024], tmpB)

        # ---- output projection ----
        with ExitStack() as ctx2:
            psE = ctx2.enter_context(
                tc.tile_pool(name=f"psE{r}", bufs=1, space="PSUM"))
            ost = ctx2.enter_context(tc.tile_pool(name=f"ost{r}", bufs=2))
            for tq in range(4):
                stg = ost.tile([128, 4, D], F32, tag="ost", name=f"stg{tq}")
                ps = psE.tile([128, 4, D], F32, tag="psE", name="pse")
                for tt in range(4):
                    t = tq * 4 + tt
                    for nt in range(2):
                        for pr in range(2):
                            nc.tensor.matmul(
                                ps[:, tt, nt * 512:(nt + 1) * 512],
                                oTp[pr][:, t * 128:(t + 1) * 128],
                                wo[:, pr, nt * 512:(nt + 1) * 512],
                                start=(pr == 0), stop=(pr == 1))
                nc.vector.tensor_copy(stg, ps)
                nc.sync.dma_start(
                    bass.AP(tensor=out_t, offset=tq * 512 * D,
                            ap=[[D, 128], [128 * D, 4], [1, D]]),
                    stg)


_PROGRAM_CACHE = {}


def get_program(apply_gamma_beta: bool, reps: int = 1):
    key = (apply_gamma_beta, reps)
    if key not in _PROGRAM_CACHE:
        _PROGRAM_CACHE[key] = build_program_v2(apply_gamma_beta, reps)
    return _PROGRAM_CACHE[key]


def shard_inputs(x, ln_gamma, ln_beta, w_qkv, w_out):
    """Build the 8 per-core input maps."""
    x = np.asarray(x, dtype=np.float32)
    ln_gamma = np.asarray(ln_gamma, dtype=np.float32)
    ln_beta = np.asarray(ln_beta, dtype=np.float32)
    w_qkv = np.asarray(w_qkv, dtype=np.float32)
    w_out = np.asarray(w_out, dtype=np.float32)
    inner = HEADS * DH
    in_maps = []
    for d in range(8):
        bi, hg = divmod(d, 4)
        c0 = hg * M_LOC
        wq = w_qkv[:, c0:c0 + M_LOC]
        wk = w_qkv[:, inner + c0:inner + c0 + M_LOC]
        wv = w_qkv[:, 2 * inner + c0:2 * inner + c0 + M_LOC]
        m = {
            "wqk_s": np.ascontiguousarray(np.concatenate([wq, wk], axis=1)),
            "wv_s": np.ascontiguousarray(wv),
            "wo_s": np.ascontiguousarray(w_out[c0:c0 + M_LOC, :]),
            "gamma_s": ln_gamma,
            "beta_s": ln_beta,
        }
        m["xt_s"] = np.ascontiguousarray(x[bi].T)
        in_maps.append(m)
    return in_maps


def unshard_outputs(results):
    """results: list of 8 dicts with 'out_s' -> full [B, N, D] output."""
    out = np.zeros((B, N, D), dtype=np.float32)
    for d in range(8):
        bi = d // 4
        out[bi] += results[d]["out_s"]
    return out


def kernel(x, ln_gamma, ln_beta, w_qkv, w_out):
    from concourse import bass_utils

    ln_gamma = np.asarray(ln_gamma, dtype=np.float32)
    ln_beta = np.asarray(ln_beta, dtype=np.float32)
    apply_gb = not (np.all(ln_gamma == 1.0) and np.all(ln_beta == 0.0))
    nc = get_program(apply_gb)
    in_maps = shard_inputs(x, ln_gamma, ln_beta, w_qkv, w_out)
    res = bass_utils.run_bass_kernel_spmd(nc, in_maps, core_ids=list(range(8)))
    return unshard_outputs(res.results)

